# revision 1
# baseline (speedup 1.0000x reference)
"""Trainium2 Bass kernel for nn_Beam_Search_Tree (moe_routing).

Strategy (pure data parallel over 8 NeuronCores):
 - Host folds all per-node PhaseShifter weights + the leaf DFT codebook into a
   single real matrix Wbig [128, 256]. For every tree node j (63 nodes total:
   1+2+4+8+16+32) it holds 4 columns u,s,v,t such that for the complex channel
   h = x[:64] + i*x[64:]:
       u = Re(y0-y1), s = Im(y0-y1), v = Re(y0+y1), t = Im(y0+y1)
   where y_k = h . w_k of the node's two children. Then the per-node softmax
   gain difference is d = |y0|^2 - |y1|^2 = u*v + s*t and the child
   probabilities are sigmoid(+-d).
 - Host transposes each core's batch shard (p-major chunk interleave) so the
   PE stationary operand (lhsT = x^T chunk) loads directly from SBUF with no
   on-device transpose, and output rows of one partition are contiguous in
   DRAM (big DMA bursts both directions).
 - Probability-tree storage uses a bit-reversal "grouped" order per layer
   (P_{l+1} = [child0-block | child1-block]) so every tree update writes a
   contiguous fp16 block (DVE 2x perf mode); the host unpermutes the 64 beam
   columns at the end. Output is fp16 on device (halves the store traffic),
   converted to f32 on host.
 - Device per 128-row chunk: one float32r matmul [128f x 128b]^T @ [128f, 256]
   -> y in PSUM; ACT copies the v/t half to SBUF + computes both sigmoids
   (fp16 out); DVE forms u*v, s*t and the fp16 tree products; GPSIMD adds
   d = m1 + m2.
"""

import sys
import numpy as np

if '/opt/trn_rl_repo' not in sys.path:
    sys.path.insert(0, '/opt/trn_rl_repo')

N_ANT = 64
N_BEAM = 64
N_CORES = 8
BATCH = 131072
B_SHARD = BATCH // N_CORES       # 16384
CHUNK = 128
N_CHUNKS = B_SHARD // CHUNK      # 128

CFG = dict(
    sg_schedule=(8, 8, 16, 24, 24, 24, 16, 8),  # chunks per super-group
    pb=8,            # chunks per PSUM tile
    ld_chunks=16,     # chunks per input dma
    psum_bufs=2,
    xt_bufs=3,
    out_fp16=True,
    d_add_eng="gpsimd",     # vector | gpsimd
    tree_eng=("vector",) * 5,  # layers 1..5
    p1_eng="scalar",        # scalar (sigmoid(-d)) | vector (1-p0 fp16 4x)
    l0_eng="vector",
    vt_bufs=8,
    md_bufs=4,
    d_bufs=4,
    p_bufs=4,
    tree_bufs=3,
    out_bufs=3,
    pipe_depth=1,
    po_sub=True,
    tree_merge=False,
    d_fp16=True,
    pb_first=8,
    pe_warm=20,
    ld_first=16,
    out_group=1,
    in_dma_engs=("sync",),            # alternates per load
    out_dma_engs=("scalar",),         # alternates per store
    skip_products=False,
    skip_tree=False,
)

# layer l block of the 64-wide d/p vectors starts at OFFS[l] (all even, so
# every fp16 slice is 4-byte aligned for the DVE 2x mode)
OFFS = [0, 2, 4, 8, 16, 32]
NS = [1, 2, 4, 8, 16, 32]

_compiled_nc = None


def configure(**kw):
    global _compiled_nc
    CFG.update(kw)
    _compiled_nc = None


def _pi_orders():
    """Grouped (bit-reversal) storage orders. pis[l][i] = tree-node index of
    the layer-l node stored at position i. pi6[j] = beam index of device
    output column j."""
    pis = [[0]]
    for _ in range(5):
        prev = pis[-1]
        pis.append([2 * k for k in prev] + [2 * k + 1 for k in prev])
    pi6 = [2 * k for k in pis[5]] + [2 * k + 1 for k in pis[5]]
    return pis, pi6


def build_wbig(thetas):
    """[128, 256] f32: blocks [U(64) | S(64) | V(64) | T(64)]; within each
    block, layer l occupies columns [OFFS[l], OFFS[l]+NS[l]) in grouped
    (bit-reversal) node order."""
    inv = 1.0 / np.sqrt(N_ANT)
    pis, _ = _pi_orders()
    layer_pairs = []  # layer_pairs[l][k] = (w0, w1) for tree node k
    for l in range(5):
        th = np.asarray(thetas[l], dtype=np.float64)      # (2^l, 64, 2)
        W = np.exp(1j * th) * inv
        layer_pairs.append([(W[i, :, 0], W[i, :, 1]) for i in range(th.shape[0])])
    az = np.arccos(np.linspace(np.cos(0.0), np.cos(np.pi - 1e-6), N_BEAM))
    A = np.exp(1j * np.pi * np.outer(np.arange(N_ANT), np.cos(az))) / np.sqrt(N_ANT)
    layer_pairs.append([(A[:, 2 * i], A[:, 2 * i + 1]) for i in range(N_BEAM // 2)])

    Wbig = np.zeros((128, 256), np.float32)
    for l in range(6):
        for i in range(NS[l]):
            w0, w1 = layer_pairs[l][pis[l][i]]
            j = OFFS[l] + i
            D = w0 - w1
            Sm = w0 + w1
            Wbig[:, j] = np.concatenate([D.real, -D.imag])           # U
            Wbig[:, 64 + j] = np.concatenate([D.imag, D.real])       # S
            Wbig[:, 128 + j] = np.concatenate([Sm.real, -Sm.imag])   # V
            Wbig[:, 192 + j] = np.concatenate([Sm.imag, Sm.real])    # T
    return Wbig


def _build():
    from concourse import bacc, mybir
    import concourse.tile as tile
    from contextlib import ExitStack

    F32 = mybir.dt.float32
    F32R = mybir.dt.float32r
    F16 = mybir.dt.float16
    AF = mybir.ActivationFunctionType
    PB = CFG["pb"]
    LD = CFG["ld_chunks"]
    SGS = CFG["sg_schedule"]
    assert sum(SGS) == N_CHUNKS
    OUT_DT = F16 if CFG["out_fp16"] else F32

    nc = bacc.Bacc("TRN2", target_bir_lowering=False, debug=False)
    xt_d = nc.dram_tensor("xt", (128, B_SHARD), F32R, kind="ExternalInput").ap()
    w_d = nc.dram_tensor("w", (128, 256), F32R, kind="ExternalInput").ap()
    out_d = nc.dram_tensor("out", (B_SHARD, 64), OUT_DT, kind="ExternalOutput").ap()
    # host uses p-major interleave: DRAM row (p*N_CHUNKS + c) <-> chunk c, partition p
    out_v = out_d.rearrange("(p c) j -> p c j", c=N_CHUNKS)   # [128, N_CHUNKS, 64]

    with tile.TileContext(nc) as tc:
        with ExitStack() as ctx:
            const = ctx.enter_context(tc.tile_pool(name="const", bufs=1))
            xtp = ctx.enter_context(tc.tile_pool(name="xtp", bufs=CFG["xt_bufs"]))
            psp = ctx.enter_context(tc.tile_pool(name="psp", bufs=CFG["psum_bufs"], space="PSUM"))
            vtp = ctx.enter_context(tc.tile_pool(name="vtp", bufs=CFG["vt_bufs"]))
            mdp = ctx.enter_context(tc.tile_pool(name="mdp", bufs=CFG["md_bufs"]))
            dp = ctx.enter_context(tc.tile_pool(name="dpool", bufs=CFG["d_bufs"]))
            pp = ctx.enter_context(tc.tile_pool(name="ppool", bufs=CFG["p_bufs"]))
            trp = ctx.enter_context(tc.tile_pool(name="tree", bufs=CFG["tree_bufs"]))
            outp = ctx.enter_context(tc.tile_pool(name="outp", bufs=CFG["out_bufs"]))

            w_sb = const.tile([128, 256], F32R)
            nc.sync.dma_start(out=w_sb[:], in_=w_d)

            # warm the ACT function tables (Sigmoid + Copy) so the
            # LoadActFuncSet overlaps the first input DMA
            warm = const.tile([128, 2], F32)
            nc.vector.memset(warm[:], 0.0)
            warm16 = const.tile([128, 2], F16)
            nc.scalar.activation(warm16[:], warm[:], AF.Sigmoid)
            nc.scalar.copy(warm[:, 0:1], warm[:, 1:2])

            # warm the PE (HAM p-state ramp) with dummy matmuls on the weight
            # tile while the first input load is still in flight
            if CFG["pe_warm"]:
                wp = psp.tile([128, PB, 256], F32, name="warm_ps", tag="y")
                for i in range(CFG["pe_warm"]):
                    nc.tensor.matmul(wp[:, i % PB, :], w_sb[:, 0:128], w_sb[:],
                                     start=True, stop=True)

            dma_counts = [0, 0]

            def in_eng():
                engs = CFG["in_dma_engs"]
                e = engs[dma_counts[0] % len(engs)]
                dma_counts[0] += 1
                return getattr(nc, e)

            def out_eng():
                engs = CFG["out_dma_engs"]
                e = engs[dma_counts[1] % len(engs)]
                dma_counts[1] += 1
                return getattr(nc, e)

            def stage_a(c_lo, SG_CHUNKS):
                first = (c_lo == 0)
                LDe = CFG["ld_first"] if first else LD
                xt = xtp.tile([128, SG_CHUNKS * CHUNK], F32R)
                for ld in range(0, SG_CHUNKS, LDe):
                    lo = ld * CHUNK
                    n_cols = min(LDe, SG_CHUNKS - ld) * CHUNK
                    in_eng().dma_start(
                        out=xt[:, lo:lo + n_cols],
                        in_=xt_d[:, c_lo * CHUNK + lo: c_lo * CHUNK + lo + n_cols],
                    )
                md = mdp.tile([128, SG_CHUNKS, 2, 64], F32, tag="md")
                PBe = min(CFG["pb_first"] if first else PB, SG_CHUNKS)
                for pt in range(SG_CHUNKS // PBe):
                    y = psp.tile([128, PBe, 256], F32)
                    for c in range(PBe):
                        col0 = (pt * PBe + c) * CHUNK
                        nc.tensor.matmul(
                            y[:, c, :], xt[:, col0:col0 + CHUNK], w_sb[:],
                            start=True, stop=True,
                        )
                    vt = vtp.tile([128, PBe, 128], F32, tag="vt")
                    nc.scalar.copy(vt[:], y[:, :, 128:256])
                    s0, s1 = pt * PBe, (pt + 1) * PBe
                    us_v = y[:, :, 0:128].rearrange("p c (two k) -> p c two k", two=2)
                    vt_v = vt[:].rearrange("p c (two k) -> p c two k", two=2)
                    nc.vector.tensor_mul(md[:, s0:s1, :, :], us_v, vt_v)

                d = dp.tile([128, SG_CHUNKS, 64], F16 if CFG["d_fp16"] else F32)
                getattr(nc, CFG["d_add_eng"]).tensor_add(
                    d[:], md[:, :, 0, :], md[:, :, 1, :])
                return (d,)

            ob_state = {"tile": None, "base": None, "filled": 0, "size": 0}

            def stage_b_merged(c_lo, SG_CHUNKS, d, flush):
                import concourse.bass as bass_mod
                pb2 = pp.tile([128, SG_CHUNKS, 2, 64], F16, tag="p0", name="pb2")
                nc.scalar.activation(pb2[:, :, 0, :], d[:], AF.Sigmoid)
                nc.scalar.activation(pb2[:, :, 1, :], d[:], AF.Sigmoid, scale=-1.0)
                Pap = pb2[:, :, :, 0:1]     # P1 = [p0(node0) | p1(node0)], [128, SG, 2, 1]
                Pn = None
                for l in range(1, 6):
                    o, n = OFFS[l], NS[l]
                    if l < 5:
                        Pn = trp.tile([128, SG_CHUNKS, 2 * n], F16, tag=f"P{l}", name=f"P{l}")
                    else:
                        Pn = outp.tile([128, SG_CHUNKS, 64], OUT_DT, name="outt")
                    out_v4 = Pn[:].rearrange("p c (two k) -> p c two k", two=2)
                    a = Pap
                    if l == 1:
                        # [128, SG, 2, 1] -> broadcast node dim: [128, SG, 2, 2]
                        in0 = bass_mod.AP(tensor=a.tensor, offset=a.offset,
                                          ap=[a.ap[0], a.ap[1], a.ap[2], [0, 2]])
                    else:
                        # flat [128, SG, n] -> broadcast pe/po dim: [128, SG, 2, n]
                        in0 = bass_mod.AP(tensor=a.tensor, offset=a.offset,
                                          ap=[a.ap[0], a.ap[1], [0, 2], a.ap[2]])
                    in1 = pb2[:, :, :, o:o + n]
                    nc.vector.tensor_mul(out_v4, in0, in1)
                    Pap = Pn[:]
                out_eng().dma_start(out=out_v[:, c_lo:c_lo + SG_CHUNKS, :], in_=Pn[:])

            def stage_b(c_lo, SG_CHUNKS, d, flush):
                if CFG["tree_merge"]:
                    return stage_b_merged(c_lo, SG_CHUNKS, d, flush)
                p0 = pp.tile([128, SG_CHUNKS, 64], F16, tag="p0")
                nc.scalar.activation(p0[:], d[:], AF.Sigmoid)
                if CFG["po_sub"]:
                    p1 = None
                else:
                    p1 = pp.tile([128, SG_CHUNKS, 64], F16, tag="p1")
                    if CFG["p1_eng"] == "scalar":
                        nc.scalar.activation(p1[:], d[:], AF.Sigmoid, scale=-1.0)
                    else:
                        nc.vector.tensor_scalar(p1[:], p0[:], -1.0, 1.0,
                                                mybir.AluOpType.mult, mybir.AluOpType.add)
                if CFG["skip_tree"]:
                    Pn = outp.tile([128, SG_CHUNKS, 64], OUT_DT)
                    nc.vector.tensor_copy(Pn[:, :, 0:64], p0[:, :, 0:64])
                    out_eng().dma_start(out=out_v[:, c_lo:c_lo + SG_CHUNKS, :], in_=Pn[:])
                    return
                P = None
                for l in range(6):
                    o, n = OFFS[l], NS[l]
                    if l < 5:
                        Pn = trp.tile([128, SG_CHUNKS, 2 * n], F16, tag=f"P{l}")
                        pe = Pn[:, :, 0:n]
                        po = Pn[:, :, n:2 * n]
                    else:
                        if ob_state["tile"] is None:
                            gsz = SG_CHUNKS
                            j = sg_index[0]
                            for k in range(1, CFG["out_group"]):
                                if j + k < len(SGS):
                                    gsz += SGS[j + k]
                            ob_state.update(tile=outp.tile([128, gsz, 64], OUT_DT, tag="outg", name="outg"),
                                            base=c_lo, filled=0, size=gsz)
                        f0 = ob_state["filled"]
                        Pn = ob_state["tile"][:, f0:f0 + SG_CHUNKS, :]
                        pe = Pn[:, :, 0:n]
                        po = Pn[:, :, n:2 * n]
                    leng = getattr(nc, CFG["l0_eng"])
                    if l == 0:
                        leng.tensor_copy(pe, p0[:, :, 0:1])
                        if CFG["po_sub"]:
                            leng.tensor_scalar(po, p0[:, :, 0:1], -1.0, 1.0,
                                               mybir.AluOpType.mult, mybir.AluOpType.add)
                        else:
                            leng.tensor_copy(po, p1[:, :, 0:1])
                    else:
                        teng = getattr(nc, CFG["tree_eng"][l - 1])
                        Pap = P[:]
                        teng.tensor_mul(pe, Pap, p0[:, :, o:o + n])
                        if CFG["po_sub"]:
                            teng.tensor_sub(po, Pap, pe)
                        else:
                            teng.tensor_mul(po, Pap, p1[:, :, o:o + n])
                    P = Pn
                ob_state["filled"] += SG_CHUNKS
                if flush or ob_state["filled"] == ob_state["size"]:
                    b0, nfill = ob_state["base"], ob_state["filled"]
                    out_eng().dma_start(
                        out=out_v[:, b0:b0 + nfill, :],
                        in_=ob_state["tile"][:, 0:nfill, :],
                    )
                    ob_state["tile"] = None

            # software pipeline: emit SG s's tree a few SGs later so the
            # tree inputs are ready when the in-order engines reach those
            # instructions.
            DEPTH = CFG["pipe_depth"]
            pend = []
            c_lo = 0
            sg_index = [0]
            n_done = [0]

            def run_b(t):
                sg_index[0] = n_done[0]
                stage_b(*t, flush=(n_done[0] == len(SGS) - 1))
                n_done[0] += 1

            for sg, SG_CHUNKS in enumerate(SGS):
                d_out = stage_a(c_lo, SG_CHUNKS)
                pend.append((c_lo, SG_CHUNKS, *d_out))
                if len(pend) > DEPTH:
                    run_b(pend.pop(0))
                c_lo += SG_CHUNKS
            for t in pend:
                run_b(t)
    nc.compile()
    return nc


def _get_nc():
    global _compiled_nc
    if _compiled_nc is None:
        _compiled_nc = _build()
    return _compiled_nc


def _shard_host(xbatch):
    """x shard [16384, 128] -> xT [128, 16384] with p-major column order:
    xt column (c*128 + m) = x row (m*N_CHUNKS + c), i.e. matmul chunk c puts
    batch row (m*N_CHUNKS + c) on output partition m, and the out DRAM row
    index p*N_CHUNKS + c equals the batch row."""
    x3 = xbatch.reshape(128, N_CHUNKS, 128)       # [m, c, f]
    return np.ascontiguousarray(x3.transpose(2, 1, 0).reshape(128, B_SHARD))


def run_sharded(xbatch, thetas, **run_kwargs):
    """Returns (out [BATCH, 64] f32, BassKernelResults)."""
    from concourse import bass_utils

    nc = _get_nc()
    xbatch = np.ascontiguousarray(np.asarray(xbatch, dtype=np.float32))
    wbig = build_wbig(thetas)
    in_maps = []
    for c in range(N_CORES):
        sh = xbatch[c * B_SHARD:(c + 1) * B_SHARD]
        in_maps.append({"xt": _shard_host(sh), "w": wbig})
    res = bass_utils.run_bass_kernel_spmd(
        nc, in_maps, core_ids=list(range(N_CORES)), **run_kwargs
    )
    _, pi6 = _pi_orders()
    pi6 = np.asarray(pi6)
    out = np.empty((BATCH, 64), np.float32)
    for c in range(N_CORES):
        o = res.results[c]["out"].astype(np.float32)
        out[c * B_SHARD:(c + 1) * B_SHARD, pi6] = o
    return out, res


def kernel(xbatch, theta0, theta1, theta2, theta3, theta4):
    out, _ = run_sharded(xbatch, [theta0, theta1, theta2, theta3, theta4])
    return out

